# revision 12
# baseline (speedup 1.0000x reference)
"""DeepRouter MoE routing kernel for 8 Trainium2 NeuronCores.

Computes, for x [16384, 4096], W [64, 4096], b [64]:
    logits = x @ W.T + b            # [T, 64]
    scores = softmax(logits, -1)
    vals, idx = top_k(scores, 8)
    weights = vals / sum(vals)      # global sum over all tokens
returns (idx.reshape(-1) int32, weights [T, 8] f32)

Sharding: data-parallel over tokens (2048 tokens/core), gate weights
replicated, one AllReduce for the global normalization sum.

Per-core device pipeline (Tile framework):
  - DMA x group [128, 4096] -> SBUF
  - PE transposes x tiles (fp32 identity transpose) -> PSUM, copied to SBUF
    (DVE/ACT alternating) to get x^T with the contraction dim on partitions
  - PE fp32 matmuls accumulate logits [128 tok, 64 expert] in PSUM
    (bias folded in via a rank-1 ones @ b matmul)
  - ACT Exp with accum_out gives p = exp(logits) and s = sum_p per token
  - DVE: scores = p * (1/s); max8 -> top-8 values; find_index8 -> indices
  - PE ones-matmul accumulates sum(top8 scores) across partitions/groups
  - AllReduce[1,8] across the 8 cores; weights = scores_top8 * (1/S_global)
Host: un-permutes [p, g, k] layouts back to token order and fixes the rare
find_index8 duplicate-needle collapse using the dumped score matrix.
"""

import sys

for _p in ("/opt/trn_rl_repo", "/root/.axon_site/_ro/trn_rl_repo"):
    if _p not in sys.path:
        sys.path.append(_p)

import numpy as np

import concourse.bass as bass
import concourse.bacc as bacc
import concourse.tile as tile
from concourse import mybir
from concourse import bass_utils

TOKENS = 16384
D = 4096
E = 64
TOPK = 8
CORES = 8
TL = TOKENS // CORES          # tokens per core (2048)
G = TL // 128                 # token groups of 128 per core (16)
NCH = D // 128                # contraction chunks (32)
QUADS = NCH // 4              # transpose quads (8)

F32 = mybir.dt.float32
U32 = mybir.dt.uint32


def _build_kernel():
    nc = bacc.Bacc(
        trn_type="TRN2",
        target_bir_lowering=False,
        debug=False,
        num_devices=CORES,
    )

    x = nc.dram_tensor("x", [TL, D], F32, kind="ExternalInput").ap()
    wt = nc.dram_tensor("wt", [128, NCH * E], F32, kind="ExternalInput").ap()
    bias = nc.dram_tensor("bias", [1, E], F32, kind="ExternalInput").ap()
    ident = nc.dram_tensor("ident", [128, 128], F32, kind="ExternalInput").ap()

    out_idx = nc.dram_tensor("out_idx", [128, G * TOPK], U32, kind="ExternalOutput").ap()
    out_w = nc.dram_tensor("out_w", [128, G * TOPK], F32, kind="ExternalOutput").ap()
    out_p = nc.dram_tensor("out_p", [128, G * E], F32, kind="ExternalOutput").ap()

    with tile.TileContext(nc) as tc:
        _kernel_body(tc, x, wt, bias, ident, out_idx, out_w, out_p)
    nc.compile()
    return nc


def _kernel_body(tc, x, wt, bias, ident, out_idx, out_w, out_p):
    nc = tc.nc
    Exp = mybir.ActivationFunctionType.Exp

    with (
        tc.tile_pool(name="singles", bufs=1) as singles,
        tc.tile_pool(name="xg_pool", bufs=3) as xg_pool,
        tc.tile_pool(name="xt_pool", bufs=3) as xt_pool,
        tc.tile_pool(name="ptmp_pool", bufs=2) as ptmp_pool,
        tc.tile_pool(name="small_pool", bufs=4) as small_pool,
        tc.tile_pool(name="tp_pool", bufs=3, space="PSUM") as tp_pool,
        tc.tile_pool(name="lg_pool", bufs=2, space="PSUM") as lg_pool,
        tc.tile_pool(name="acc_pool", bufs=1, space="PSUM") as acc_pool,
        tc.tile_pool(name="dram", bufs=1, space="DRAM") as dram,
    ):
        # constants — small DMAs first so the pipeline can start quickly
        ident_sb = singles.tile([128, 128], F32)
        nc.sync.dma_start(out=ident_sb, in_=ident)
        # per-expert bias broadcast across all 128 partitions
        b_bcast = singles.tile([128, E], F32)
        b_bc_ap = bass.AP(tensor=bias.tensor, offset=bias.offset,
                          ap=[[0, 128], [1, E]])
        nc.gpsimd.dma_start(out=b_bcast, in_=b_bc_ap)

        ones_row = singles.tile([1, 128], F32)
        nc.vector.memset(ones_row, 1.0)
        ones_col = singles.tile([128, 1], F32)
        nc.vector.memset(ones_col, 1.0)

        # pre-warm the ACT exp table so the ~2.7us table load overlaps startup
        warm = singles.tile([1, 1], F32)
        nc.scalar.activation(out=warm, in_=ones_row[:, 0:1],
                             func=mybir.ActivationFunctionType.Exp)

        p_all = singles.tile([128, G * E], F32)      # scores dump
        vals_all = singles.tile([128, G * TOPK], F32)
        idx_all = singles.tile([128, G * TOPK], U32)

        sum_ps = acc_pool.tile([1, TOPK], F32)       # global-sum accumulator

        # first group's first chunk before the big weight tensor
        xg0 = xg_pool.tile([128, D], F32)
        nc.sync.dma_start(out=xg0[:, 0:512], in_=x[0:128, 0:512])
        wt_sb = singles.tile([128, NCH * E], F32)
        nc.sync.dma_start(out=wt_sb, in_=wt)

        for g in range(G):
            if g == 0:
                xg = xg0
                for q in range(1, QUADS):
                    nc.sync.dma_start(out=xg[:, q * 512:(q + 1) * 512],
                                      in_=x[0:128, q * 512:(q + 1) * 512])
            else:
                xg = xg_pool.tile([128, D], F32)
                for q in range(QUADS):
                    nc.sync.dma_start(
                        out=xg[:, q * 512:(q + 1) * 512],
                        in_=x[g * 128:(g + 1) * 128, q * 512:(q + 1) * 512])

            lg = lg_pool.tile([128, E], F32)
            for q in range(QUADS):
                tp = tp_pool.tile([128, 512], F32)
                for j in range(4):
                    i = q * 4 + j
                    nc.tensor.transpose(
                        tp[:, j * 128:(j + 1) * 128],
                        xg[:, i * 128:(i + 1) * 128],
                        ident_sb,
                    )
                xt = xt_pool.tile([128, 512], F32)
                if q % 2 == 0:
                    nc.vector.tensor_copy(xt, tp)
                else:
                    nc.scalar.copy(xt, tp)
                for j in range(4):
                    i = q * 4 + j
                    nc.tensor.matmul(
                        lg,
                        lhsT=xt[:, j * 128:(j + 1) * 128],
                        rhs=wt_sb[:, i * E:(i + 1) * E],
                        start=(i == 0),
                        stop=(i == NCH - 1),
                        skip_group_check=True,
                    )

            # epilogue for this group of 128 tokens
            nc.vector.tensor_add(lg, lg, b_bcast)    # logits += b (in PSUM)
            p_tmp = ptmp_pool.tile([128, E], F32)
            s_tok = small_pool.tile([128, 1], F32)
            nc.scalar.activation(out=p_tmp, in_=lg, func=Exp, bias=0.0, scale=1.0,
                                 accum_out=s_tok)
            r_tok = small_pool.tile([128, 1], F32)
            nc.vector.reciprocal(r_tok, s_tok)

            sc = p_all[:, g * E:(g + 1) * E]
            nc.vector.tensor_scalar_mul(sc, p_tmp, r_tok)

            pv = vals_all[:, g * TOPK:(g + 1) * TOPK]
            nc.vector.max(out=pv, in_=sc)
            nc.vector.max_index(out=idx_all[:, g * TOPK:(g + 1) * TOPK],
                                in_max=pv, in_values=sc)

            # accumulate sum over partitions (tokens) of the top-8 scores
            nc.tensor.matmul(sum_ps, lhsT=ones_col, rhs=pv,
                             start=(g == 0), stop=(g == G - 1),
                             skip_group_check=True)

        # idx / score-dump outputs don't depend on the collective — flush now
        nc.sync.dma_start(out=out_idx, in_=idx_all)
        nc.sync.dma_start(out=out_p, in_=p_all)

        # ---- global sum across cores ----
        loc8 = singles.tile([1, TOPK], F32)
        nc.vector.tensor_copy(loc8, sum_ps)
        cc_in = dram.tile([1, TOPK], F32)
        cc_out = dram.tile([1, TOPK], F32)
        nc.sync.dma_start(out=cc_in, in_=loc8)
        nc.gpsimd.collective_compute(
            "AllReduce",
            mybir.AluOpType.add,
            replica_groups=[list(range(CORES))],
            ins=[cc_in.opt()],
            outs=[cc_out.opt()],
        )
        glob8 = singles.tile([1, TOPK], F32)
        nc.sync.dma_start(out=glob8, in_=cc_out)
        s_glob = singles.tile([1, 1], F32)
        nc.vector.reduce_sum(s_glob, glob8, axis=mybir.AxisListType.X)

        # broadcast S to all partitions via ones matmul, then scale
        bc_ps = acc_pool.tile([128, 1], F32)
        nc.tensor.matmul(bc_ps, lhsT=ones_row, rhs=s_glob, start=True, stop=True,
                         skip_group_check=True)
        s_bcast = singles.tile([128, 1], F32)
        nc.vector.tensor_copy(s_bcast, bc_ps)
        r_glob = singles.tile([128, 1], F32)
        nc.vector.reciprocal(r_glob, s_bcast)

        w_sb = singles.tile([128, G * TOPK], F32)
        nc.vector.tensor_scalar_mul(w_sb, vals_all, r_glob)

        nc.sync.dma_start(out=out_w, in_=w_sb)


_NC_CACHE = {}


def _get_nc():
    if "nc" not in _NC_CACHE:
        _NC_CACHE["nc"] = _build_kernel()
    return _NC_CACHE["nc"]


def _unpermute(a):
    """[128, G*inner] per-core layout -> [TL, inner] token-major."""
    inner = a.shape[1] // G
    return a.reshape(128, G, inner).transpose(1, 0, 2).reshape(TL, inner)


def _fix_duplicate_indices(idx, scores):
    """find_index8 latches the first matching position per needle, so tokens
    whose top-8 contains bitwise-equal scores get the same index reported
    multiple times. Reassign ascending positions (jax.lax.top_k tie order)."""
    dup_rows = np.nonzero((idx[:, 1:] == idx[:, :-1]).any(axis=1))[0]
    for t in dup_rows:
        row = idx[t]
        uniq, counts = np.unique(row, return_counts=True)
        for u, m in zip(uniq, counts):
            if m < 2:
                continue
            v = scores[t, u]
            slots = np.nonzero(row == u)[0]
            pos = np.nonzero(scores[t] == v)[0][: len(slots)]
            idx[t, slots] = pos
    return idx


def kernel(x, W, b):
    x = np.asarray(x, dtype=np.float32)
    W = np.asarray(W, dtype=np.float32)
    b = np.asarray(b, dtype=np.float32)

    # W^T retiled so each SBUF partition holds its contraction rows
    # contiguously: wt[p, i*E + e] = W[e, i*128 + p]
    wt = np.ascontiguousarray(
        W.T.reshape(NCH, 128, E).transpose(1, 0, 2).reshape(128, NCH * E)
    )
    bias = np.ascontiguousarray(b.reshape(1, E))
    ident = np.eye(128, dtype=np.float32)

    in_maps = [
        {
            "x": np.ascontiguousarray(x[c * TL:(c + 1) * TL]),
            "wt": wt,
            "bias": bias,
            "ident": ident,
        }
        for c in range(CORES)
    ]

    nc = _get_nc()
    res = bass_utils.run_bass_kernel_spmd(nc, in_maps, core_ids=list(range(CORES)))

    idx_parts = []
    w_parts = []
    for c in range(CORES):
        out = res.results[c]
        idx_c = _unpermute(out["out_idx"]).astype(np.int64)
        w_c = _unpermute(out["out_w"]).astype(np.float32)
        p_c = _unpermute(out["out_p"]).astype(np.float32)
        idx_c = _fix_duplicate_indices(idx_c, p_c)
        idx_parts.append(idx_c)
        w_parts.append(w_c)

    idx = np.concatenate(idx_parts, axis=0).astype(np.int32)
    weights = np.concatenate(w_parts, axis=0)
    return idx.reshape(-1), weights


# revision 15
# speedup vs baseline: 1.0631x; 1.0631x over previous
"""DeepRouter MoE routing kernel for 8 Trainium2 NeuronCores.

Computes, for x [16384, 4096], W [64, 4096], b [64]:
    logits = x @ W.T + b            # [T, 64]
    scores = softmax(logits, -1)
    vals, idx = top_k(scores, 8)
    weights = vals / sum(vals)      # global sum over all tokens
returns (idx.reshape(-1) int32, weights [T, 8] f32)

Sharding: data-parallel over tokens (2048 tokens/core), gate weights
replicated, one AllReduce for the global normalization sum.

Per-core device pipeline (Tile framework):
  - DMA x group [128, 4096] -> SBUF
  - PE transposes x tiles (fp32 identity transpose) -> PSUM, copied to SBUF
    (DVE/ACT alternating) to get x^T with the contraction dim on partitions
  - PE fp32 matmuls accumulate logits [128 tok, 64 expert] in PSUM
    (bias folded in via a rank-1 ones @ b matmul)
  - ACT Exp with accum_out gives p = exp(logits) and s = sum_p per token
  - DVE: scores = p * (1/s); max8 -> top-8 values; find_index8 -> indices
  - PE ones-matmul accumulates sum(top8 scores) across partitions/groups
  - AllReduce[1,8] across the 8 cores; weights = scores_top8 * (1/S_global)
Host: un-permutes [p, g, k] layouts back to token order and fixes the rare
find_index8 duplicate-needle collapse using the dumped score matrix.
"""

import sys

for _p in ("/opt/trn_rl_repo", "/root/.axon_site/_ro/trn_rl_repo"):
    if _p not in sys.path:
        sys.path.append(_p)

import numpy as np

import concourse.bass as bass
import concourse.bacc as bacc
import concourse.tile as tile
from concourse import mybir
from concourse import bass_utils

TOKENS = 16384
D = 4096
E = 64
TOPK = 8
CORES = 8
TL = TOKENS // CORES          # tokens per core (2048)
G = TL // 128                 # token groups of 128 per core (16)
NCH = D // 128                # contraction chunks (32)
QUADS = NCH // 4              # transpose quads (8)

F32 = mybir.dt.float32
U32 = mybir.dt.uint32


def _build_kernel():
    nc = bacc.Bacc(
        trn_type="TRN2",
        target_bir_lowering=False,
        debug=False,
        num_devices=CORES,
    )

    x = nc.dram_tensor("x", [TL, D], F32, kind="ExternalInput").ap()
    wt = nc.dram_tensor("wt", [128, NCH * E], F32, kind="ExternalInput").ap()
    bias = nc.dram_tensor("bias", [1, E], F32, kind="ExternalInput").ap()
    ident = nc.dram_tensor("ident", [128, 128], F32, kind="ExternalInput").ap()

    out_idx = nc.dram_tensor("out_idx", [128, G * TOPK], U32, kind="ExternalOutput").ap()
    out_w = nc.dram_tensor("out_w", [128, G * TOPK], F32, kind="ExternalOutput").ap()
    out_p = nc.dram_tensor("out_p", [128, G * E], F32, kind="ExternalOutput").ap()

    with tile.TileContext(nc) as tc:
        _kernel_body(tc, x, wt, bias, ident, out_idx, out_w, out_p)
    nc.compile()
    return nc


def _kernel_body(tc, x, wt, bias, ident, out_idx, out_w, out_p):
    nc = tc.nc
    Exp = mybir.ActivationFunctionType.Exp

    with (
        tc.tile_pool(name="singles", bufs=1) as singles,
        tc.tile_pool(name="xg_pool", bufs=3) as xg_pool,
        tc.tile_pool(name="xt_pool", bufs=3) as xt_pool,
        tc.tile_pool(name="ptmp_pool", bufs=2) as ptmp_pool,
        tc.tile_pool(name="small_pool", bufs=4) as small_pool,
        tc.tile_pool(name="tp_pool", bufs=4, space="PSUM") as tp_pool,
        tc.tile_pool(name="lg_pool", bufs=2, space="PSUM") as lg_pool,
        tc.tile_pool(name="acc_pool", bufs=1, space="PSUM") as acc_pool,
        tc.tile_pool(name="dram", bufs=1, space="DRAM") as dram,
    ):
        # constants — small DMAs first so the pipeline can start quickly
        ident_sb = singles.tile([128, 128], F32)
        nc.sync.dma_start(out=ident_sb, in_=ident)
        # per-expert bias broadcast across all 128 partitions
        b_bcast = singles.tile([128, E], F32)
        b_bc_ap = bass.AP(tensor=bias.tensor, offset=bias.offset,
                          ap=[[0, 128], [1, E]])
        nc.gpsimd.dma_start(out=b_bcast, in_=b_bc_ap)

        ones_row = singles.tile([1, 128], F32)
        nc.vector.memset(ones_row, 1.0)
        ones_col = singles.tile([128, 1], F32)
        nc.vector.memset(ones_col, 1.0)

        # pre-warm the ACT exp table so the ~2.7us table load overlaps startup
        warm = singles.tile([1, 1], F32)
        nc.scalar.activation(out=warm, in_=ones_row[:, 0:1],
                             func=mybir.ActivationFunctionType.Exp)

        p_all = singles.tile([128, G * E], F32)      # scores dump
        vals_all = singles.tile([128, G * TOPK], F32)
        idx_all = singles.tile([128, G * TOPK], U32)

        sum_ps = acc_pool.tile([1, TOPK], F32)       # global-sum accumulator

        # first group's first chunk before the big weight tensor
        xg0 = xg_pool.tile([128, D], F32)
        nc.sync.dma_start(out=xg0[:, 0:512], in_=x[0:128, 0:512])
        wt_sb = singles.tile([128, NCH * E], F32)
        nc.sync.dma_start(out=wt_sb, in_=wt)

        for g in range(G):
            if g == 0:
                xg = xg0
                for q in range(1, QUADS):
                    nc.sync.dma_start(out=xg[:, q * 512:(q + 1) * 512],
                                      in_=x[0:128, q * 512:(q + 1) * 512])
            else:
                xg = xg_pool.tile([128, D], F32)
                nc.sync.dma_start(out=xg, in_=x[g * 128:(g + 1) * 128, :])

            lg = lg_pool.tile([128, E], F32)
            for q in range(QUADS):
                tp = tp_pool.tile([128, 512], F32)
                for j in range(4):
                    i = q * 4 + j
                    nc.tensor.transpose(
                        tp[:, j * 128:(j + 1) * 128],
                        xg[:, i * 128:(i + 1) * 128],
                        ident_sb,
                    )
                xt = xt_pool.tile([128, 512], F32)
                if q % 2 == 0:
                    nc.vector.tensor_copy(xt, tp)
                else:
                    nc.scalar.copy(xt, tp)
                for j in range(4):
                    i = q * 4 + j
                    nc.tensor.matmul(
                        lg,
                        lhsT=xt[:, j * 128:(j + 1) * 128],
                        rhs=wt_sb[:, i * E:(i + 1) * E],
                        start=(i == 0),
                        stop=(i == NCH - 1),
                        skip_group_check=True,
                    )

            # epilogue for this group of 128 tokens
            nc.vector.tensor_add(lg, lg, b_bcast)    # logits += b (in PSUM)
            p_tmp = ptmp_pool.tile([128, E], F32)
            s_tok = small_pool.tile([128, 1], F32)
            nc.scalar.activation(out=p_tmp, in_=lg, func=Exp, bias=0.0, scale=1.0,
                                 accum_out=s_tok)
            r_tok = small_pool.tile([128, 1], F32)
            nc.vector.reciprocal(r_tok, s_tok)

            sc = p_all[:, g * E:(g + 1) * E]
            nc.vector.tensor_scalar_mul(sc, p_tmp, r_tok)

            pv = vals_all[:, g * TOPK:(g + 1) * TOPK]
            nc.vector.max(out=pv, in_=sc)
            nc.vector.max_index(out=idx_all[:, g * TOPK:(g + 1) * TOPK],
                                in_max=pv, in_values=sc)

            # accumulate sum over partitions (tokens) of the top-8 scores
            nc.tensor.matmul(sum_ps, lhsT=ones_col, rhs=pv,
                             start=(g == 0), stop=(g == G - 1),
                             skip_group_check=True)

        # ---- global sum across cores ----
        loc8 = singles.tile([1, TOPK], F32)
        nc.vector.tensor_copy(loc8, sum_ps)
        cc_in = dram.tile([1, TOPK], F32)
        cc_out = dram.tile([1, TOPK], F32)
        nc.sync.dma_start(out=cc_in, in_=loc8)
        # idx / score-dump outputs don't depend on the collective — they drain
        # on the DMA ring (behind the tiny cc_in transfer) during the mesh wait
        nc.sync.dma_start(out=out_idx, in_=idx_all)
        nc.sync.dma_start(out=out_p, in_=p_all)
        nc.gpsimd.collective_compute(
            "AllReduce",
            mybir.AluOpType.add,
            replica_groups=[list(range(CORES))],
            ins=[cc_in.opt()],
            outs=[cc_out.opt()],
        )
        glob8 = singles.tile([1, TOPK], F32)
        nc.sync.dma_start(out=glob8, in_=cc_out)
        s_glob = singles.tile([1, 1], F32)
        nc.vector.reduce_sum(s_glob, glob8, axis=mybir.AxisListType.X)

        # broadcast S to all partitions via ones matmul, then scale
        bc_ps = acc_pool.tile([128, 1], F32)
        nc.tensor.matmul(bc_ps, lhsT=ones_row, rhs=s_glob, start=True, stop=True,
                         skip_group_check=True)
        s_bcast = singles.tile([128, 1], F32)
        nc.vector.tensor_copy(s_bcast, bc_ps)
        r_glob = singles.tile([128, 1], F32)
        nc.vector.reciprocal(r_glob, s_bcast)

        w_sb = singles.tile([128, G * TOPK], F32)
        nc.vector.tensor_scalar_mul(w_sb, vals_all, r_glob)

        nc.sync.dma_start(out=out_w, in_=w_sb)


_NC_CACHE = {}


def _get_nc():
    if "nc" not in _NC_CACHE:
        _NC_CACHE["nc"] = _build_kernel()
    return _NC_CACHE["nc"]


def _unpermute(a):
    """[128, G*inner] per-core layout -> [TL, inner] token-major."""
    inner = a.shape[1] // G
    return a.reshape(128, G, inner).transpose(1, 0, 2).reshape(TL, inner)


def _fix_duplicate_indices(idx, scores):
    """find_index8 latches the first matching position per needle, so tokens
    whose top-8 contains bitwise-equal scores get the same index reported
    multiple times. Reassign ascending positions (jax.lax.top_k tie order)."""
    dup_rows = np.nonzero((idx[:, 1:] == idx[:, :-1]).any(axis=1))[0]
    for t in dup_rows:
        row = idx[t]
        uniq, counts = np.unique(row, return_counts=True)
        for u, m in zip(uniq, counts):
            if m < 2:
                continue
            v = scores[t, u]
            slots = np.nonzero(row == u)[0]
            pos = np.nonzero(scores[t] == v)[0][: len(slots)]
            idx[t, slots] = pos
    return idx


def kernel(x, W, b):
    x = np.asarray(x, dtype=np.float32)
    W = np.asarray(W, dtype=np.float32)
    b = np.asarray(b, dtype=np.float32)

    # W^T retiled so each SBUF partition holds its contraction rows
    # contiguously: wt[p, i*E + e] = W[e, i*128 + p]
    wt = np.ascontiguousarray(
        W.T.reshape(NCH, 128, E).transpose(1, 0, 2).reshape(128, NCH * E)
    )
    bias = np.ascontiguousarray(b.reshape(1, E))
    ident = np.eye(128, dtype=np.float32)

    in_maps = [
        {
            "x": np.ascontiguousarray(x[c * TL:(c + 1) * TL]),
            "wt": wt,
            "bias": bias,
            "ident": ident,
        }
        for c in range(CORES)
    ]

    nc = _get_nc()
    res = bass_utils.run_bass_kernel_spmd(nc, in_maps, core_ids=list(range(CORES)))

    idx_parts = []
    w_parts = []
    for c in range(CORES):
        out = res.results[c]
        idx_c = _unpermute(out["out_idx"]).astype(np.int64)
        w_c = _unpermute(out["out_w"]).astype(np.float32)
        p_c = _unpermute(out["out_p"]).astype(np.float32)
        idx_c = _fix_duplicate_indices(idx_c, p_c)
        idx_parts.append(idx_c)
        w_parts.append(w_c)

    idx = np.concatenate(idx_parts, axis=0).astype(np.int32)
    weights = np.concatenate(w_parts, axis=0)
    return idx.reshape(-1), weights


# revision 16
# speedup vs baseline: 1.0821x; 1.0179x over previous
"""DeepRouter MoE routing kernel for 8 Trainium2 NeuronCores.

Computes, for x [16384, 4096], W [64, 4096], b [64]:
    logits = x @ W.T + b            # [T, 64]
    scores = softmax(logits, -1)
    vals, idx = top_k(scores, 8)
    weights = vals / sum(vals)      # global sum over all tokens
returns (idx.reshape(-1) int32, weights [T, 8] f32)

Sharding: data-parallel over tokens (2048 tokens/core), gate weights
replicated, one AllReduce for the global normalization sum.

Per-core device pipeline (Tile framework):
  - DMA x group [128, 4096] -> SBUF
  - PE transposes x tiles (fp32 identity transpose) -> PSUM, copied to SBUF
    (DVE/ACT alternating) to get x^T with the contraction dim on partitions
  - PE fp32 matmuls accumulate logits [128 tok, 64 expert] in PSUM;
    the per-expert bias is added by one DVE op on the finished PSUM tile
  - ACT Exp with accum_out gives p = exp(logits) and s = sum_p per token
  - DVE: scores = p * (1/s); max8 -> top-8 values; find_index8 -> indices
  - PE ones-matmul accumulates sum(top8 scores) across partitions/groups
  - AllReduce[1,8] across the 8 cores; weights = scores_top8 * (1/S_global)
Host: un-permutes [p, g, k] layouts back to token order and fixes the rare
find_index8 duplicate-needle collapse using the dumped score matrix.
"""

import sys

for _p in ("/opt/trn_rl_repo", "/root/.axon_site/_ro/trn_rl_repo"):
    if _p not in sys.path:
        sys.path.append(_p)

import numpy as np

import concourse.bass as bass
import concourse.bacc as bacc
import concourse.tile as tile
from concourse import mybir
from concourse import bass_utils

TOKENS = 16384
D = 4096
E = 64
TOPK = 8
CORES = 8
TL = TOKENS // CORES          # tokens per core (2048)
G = TL // 128                 # token groups of 128 per core (16)
NCH = D // 128                # contraction chunks (32)
QUADS = NCH // 4              # transpose quads (8)

F32 = mybir.dt.float32
U32 = mybir.dt.uint32


def _build_kernel():
    nc = bacc.Bacc(
        trn_type="TRN2",
        target_bir_lowering=False,
        debug=False,
        num_devices=CORES,
    )

    x = nc.dram_tensor("x", [TL, D], F32, kind="ExternalInput").ap()
    wt = nc.dram_tensor("wt", [128, NCH * E], F32, kind="ExternalInput").ap()
    bias = nc.dram_tensor("bias", [1, E], F32, kind="ExternalInput").ap()
    ident = nc.dram_tensor("ident", [128, 128], F32, kind="ExternalInput").ap()

    out_idx = nc.dram_tensor("out_idx", [128, G * TOPK], U32, kind="ExternalOutput").ap()
    out_w = nc.dram_tensor("out_w", [128, G * TOPK], F32, kind="ExternalOutput").ap()
    out_p = nc.dram_tensor("out_p", [128, G * E], F32, kind="ExternalOutput").ap()

    with tile.TileContext(nc) as tc:
        _kernel_body(tc, x, wt, bias, ident, out_idx, out_w, out_p)
    nc.compile()
    return nc


def _kernel_body(tc, x, wt, bias, ident, out_idx, out_w, out_p):
    nc = tc.nc
    Exp = mybir.ActivationFunctionType.Exp

    with (
        tc.tile_pool(name="singles", bufs=1) as singles,
        tc.tile_pool(name="xg_pool", bufs=3) as xg_pool,
        tc.tile_pool(name="xt_pool", bufs=3) as xt_pool,
        tc.tile_pool(name="ptmp_pool", bufs=2) as ptmp_pool,
        tc.tile_pool(name="small_pool", bufs=4) as small_pool,
        tc.tile_pool(name="tp_pool", bufs=4, space="PSUM") as tp_pool,
        tc.tile_pool(name="lg_pool", bufs=2, space="PSUM") as lg_pool,
        tc.tile_pool(name="acc_pool", bufs=1, space="PSUM") as acc_pool,
        tc.tile_pool(name="dram", bufs=1, space="DRAM") as dram,
    ):
        # constants — small DMAs first so the pipeline can start quickly
        ident_sb = singles.tile([128, 128], F32)
        nc.sync.dma_start(out=ident_sb, in_=ident)
        # per-expert bias broadcast across all 128 partitions
        b_bcast = singles.tile([128, E], F32)
        b_bc_ap = bass.AP(tensor=bias.tensor, offset=bias.offset,
                          ap=[[0, 128], [1, E]])
        nc.gpsimd.dma_start(out=b_bcast, in_=b_bc_ap)

        ones_row = singles.tile([1, 128], F32)
        nc.vector.memset(ones_row, 1.0)
        ones_col = singles.tile([128, 1], F32)
        nc.vector.memset(ones_col, 1.0)

        # pre-warm the ACT exp table so the ~2.7us table load overlaps startup
        warm = singles.tile([1, 1], F32)
        nc.scalar.activation(out=warm, in_=ones_row[:, 0:1],
                             func=mybir.ActivationFunctionType.Exp)

        p_all = singles.tile([128, G * E], F32)      # scores dump
        vals_all = singles.tile([128, G * TOPK], F32)
        idx_all = singles.tile([128, G * TOPK], U32)

        sum_ps = acc_pool.tile([1, TOPK], F32)       # global-sum accumulator

        # first group's first chunk before the big weight tensor
        xg0 = xg_pool.tile([128, D], F32)
        nc.sync.dma_start(out=xg0[:, 0:512], in_=x[0:128, 0:512])
        wt_sb = singles.tile([128, NCH * E], F32)
        nc.sync.dma_start(out=wt_sb, in_=wt)

        for g in range(G):
            if g == 0:
                xg = xg0
                for q in range(1, QUADS):
                    nc.sync.dma_start(out=xg[:, q * 512:(q + 1) * 512],
                                      in_=x[0:128, q * 512:(q + 1) * 512])
            else:
                xg = xg_pool.tile([128, D], F32)
                nc.sync.dma_start(out=xg, in_=x[g * 128:(g + 1) * 128, :])

            lg = lg_pool.tile([128, E], F32)
            for q in range(QUADS):
                tp = tp_pool.tile([128, 512], F32)
                for j in range(4):
                    i = q * 4 + j
                    nc.tensor.transpose(
                        tp[:, j * 128:(j + 1) * 128],
                        xg[:, i * 128:(i + 1) * 128],
                        ident_sb,
                    )
                xt = xt_pool.tile([128, 512], F32)
                if q % 2 == 0:
                    nc.vector.tensor_copy(xt, tp)
                else:
                    nc.scalar.copy(xt, tp)
                for j in range(4):
                    i = q * 4 + j
                    nc.tensor.matmul(
                        lg,
                        lhsT=xt[:, j * 128:(j + 1) * 128],
                        rhs=wt_sb[:, i * E:(i + 1) * E],
                        start=(i == 0),
                        stop=(i == NCH - 1),
                        skip_group_check=True,
                    )

            # epilogue for this group of 128 tokens
            nc.vector.tensor_add(lg, lg, b_bcast)    # logits += b (in PSUM)
            p_tmp = ptmp_pool.tile([128, E], F32)
            s_tok = small_pool.tile([128, 1], F32)
            nc.scalar.activation(out=p_tmp, in_=lg, func=Exp, bias=0.0, scale=1.0,
                                 accum_out=s_tok)
            r_tok = small_pool.tile([128, 1], F32)
            nc.vector.reciprocal(r_tok, s_tok)

            sc = p_all[:, g * E:(g + 1) * E]
            nc.vector.tensor_scalar_mul(sc, p_tmp, r_tok)

            pv = vals_all[:, g * TOPK:(g + 1) * TOPK]
            nc.vector.max(out=pv, in_=sc)
            nc.vector.max_index(out=idx_all[:, g * TOPK:(g + 1) * TOPK],
                                in_max=pv, in_values=sc)

            # accumulate sum over partitions (tokens) of the top-8 scores
            nc.tensor.matmul(sum_ps, lhsT=ones_col, rhs=pv,
                             start=(g == 0), stop=(g == G - 1),
                             skip_group_check=True)

        # ---- global sum across cores ----
        loc8 = singles.tile([1, TOPK], F32)
        nc.vector.tensor_copy(loc8, sum_ps)
        cc_in = dram.tile([1, TOPK], F32)
        cc_out = dram.tile([1, TOPK], F32)
        nc.sync.dma_start(out=cc_in, in_=loc8)
        # idx / score-dump outputs don't depend on the collective — they drain
        # on the DMA ring (behind the tiny cc_in transfer) during the mesh wait
        nc.sync.dma_start(out=out_idx, in_=idx_all)
        nc.sync.dma_start(out=out_p, in_=p_all)
        nc.gpsimd.collective_compute(
            "AllReduce",
            mybir.AluOpType.add,
            replica_groups=[list(range(CORES))],
            ins=[cc_in.opt()],
            outs=[cc_out.opt()],
        )
        glob8 = singles.tile([1, TOPK], F32)
        nc.sync.dma_start(out=glob8, in_=cc_out)
        s_glob = singles.tile([1, 1], F32)
        nc.vector.reduce_sum(s_glob, glob8, axis=mybir.AxisListType.X)

        # broadcast S to all partitions via ones matmul, then scale
        bc_ps = acc_pool.tile([128, 1], F32)
        nc.tensor.matmul(bc_ps, lhsT=ones_row, rhs=s_glob, start=True, stop=True,
                         skip_group_check=True)
        s_bcast = singles.tile([128, 1], F32)
        nc.vector.tensor_copy(s_bcast, bc_ps)
        r_glob = singles.tile([128, 1], F32)
        nc.vector.reciprocal(r_glob, s_bcast)

        w_sb = singles.tile([128, G * TOPK], F32)
        nc.vector.tensor_scalar_mul(w_sb, vals_all, r_glob)

        nc.sync.dma_start(out=out_w, in_=w_sb)


_NC_CACHE = {}


def _get_nc():
    if "nc" not in _NC_CACHE:
        _NC_CACHE["nc"] = _build_kernel()
    return _NC_CACHE["nc"]


def _unpermute(a):
    """[128, G*inner] per-core layout -> [TL, inner] token-major."""
    inner = a.shape[1] // G
    return a.reshape(128, G, inner).transpose(1, 0, 2).reshape(TL, inner)


def _fix_duplicate_indices(idx, scores):
    """find_index8 latches the first matching position per needle, so tokens
    whose top-8 contains bitwise-equal scores get the same index reported
    multiple times. Reassign ascending positions (jax.lax.top_k tie order)."""
    dup_rows = np.nonzero((idx[:, 1:] == idx[:, :-1]).any(axis=1))[0]
    for t in dup_rows:
        row = idx[t]
        uniq, counts = np.unique(row, return_counts=True)
        for u, m in zip(uniq, counts):
            if m < 2:
                continue
            v = scores[t, u]
            slots = np.nonzero(row == u)[0]
            pos = np.nonzero(scores[t] == v)[0][: len(slots)]
            idx[t, slots] = pos
    return idx


def kernel(x, W, b):
    x = np.asarray(x, dtype=np.float32)
    W = np.asarray(W, dtype=np.float32)
    b = np.asarray(b, dtype=np.float32)

    # W^T retiled so each SBUF partition holds its contraction rows
    # contiguously: wt[p, i*E + e] = W[e, i*128 + p]
    wt = np.ascontiguousarray(
        W.T.reshape(NCH, 128, E).transpose(1, 0, 2).reshape(128, NCH * E)
    )
    bias = np.ascontiguousarray(b.reshape(1, E))
    ident = np.eye(128, dtype=np.float32)

    in_maps = [
        {
            "x": np.ascontiguousarray(x[c * TL:(c + 1) * TL]),
            "wt": wt,
            "bias": bias,
            "ident": ident,
        }
        for c in range(CORES)
    ]

    nc = _get_nc()
    res = bass_utils.run_bass_kernel_spmd(nc, in_maps, core_ids=list(range(CORES)))

    idx_parts = []
    w_parts = []
    for c in range(CORES):
        out = res.results[c]
        idx_c = _unpermute(out["out_idx"]).astype(np.int64)
        w_c = _unpermute(out["out_w"]).astype(np.float32)
        p_c = _unpermute(out["out_p"]).astype(np.float32)
        idx_c = _fix_duplicate_indices(idx_c, p_c)
        idx_parts.append(idx_c)
        w_parts.append(w_c)

    idx = np.concatenate(idx_parts, axis=0).astype(np.int32)
    weights = np.concatenate(w_parts, axis=0)
    return idx.reshape(-1), weights
